# revision 1
# baseline (speedup 1.0000x reference)
"""DeepFM (nn_DeepFM_77120432766994) Trainium2 Bass kernel.

Strategy: data-parallel over batch across 8 NeuronCores; the embedding
table is replicated in each core's HBM.  Per core (2048 batch rows):

  host prep:
    - combined table [V, 33] f32: 32 emb dims + first-order bias column.
    - indices pre-offset (global rows), int32, permuted to gather order.
    - BN folded into MLP weights; weights pre-transposed/packed into one
      [128, 1192] f32r blob (one DMA); f32 aux (identity, biases) in a
      second blob.  Dense features pre-transposed on host.
  device, per 512-batch tile:
    - ONE indirect DMA gathers 5120 rows of 132B into G [128b, 4s, 330f]
      (feature layout: field f at cols 33f..33f+32 = emb dims + bias).
    - 12 PE transposes -> X chunks [128feat, 512batch] (features on
      partitions); dense rows 74:90 of chunk 2 arrive via DVE copy from a
      host-transposed staging tile; rows 90:128 stay garbage and are
      excluded by K=90 partition slices in all chunk-2 matmuls.
    - FM: masked matmuls: S[e,b] (field sums; bias-sum in row 32) and
      -0.5*sum(x^2) via squared X; 0.5*||s||^2 via ACT square + matmul.
    - MLP: matmul chains with fused bias+ReLU on the scalar engine.
    - Everything accumulates into one [1,512] PSUM bank -> sigmoid -> y.

  NOTE: PE matmult instructions only support a single sync-wait in
  codegen, so the dataflow is arranged to keep every PE instruction's
  newly-required semaphore count at <= 1 (single weight DMA + absorber
  transposes at t=0; X tiles written exclusively by DVE).
"""

import os
import sys

import numpy as np

for _p in ("/opt/trn_rl_repo", "/root/.axon_site/_ro/trn_rl_repo"):
    if os.path.isdir(_p) and _p not in sys.path:
        sys.path.insert(0, _p)

import concourse.bass as bass
import concourse.bacc as bacc
import concourse.mybir as mybir
import concourse.tile as tile
from concourse.bass_utils import run_bass_kernel_spmd

# ---------------------------------------------------------------- constants
FIELD_DIMS = [1000000, 100000, 1008, 1004, 102, 1000, 500, 50, 112, 107]
NF = 10
B = 16384
EMB = 32
DENSE = 16
MLP = [256, 128, 64]
BN_EPS = 1e-5
V = int(np.sum(FIELD_DIMS))
OFFSETS = np.concatenate([[0], np.cumsum(FIELD_DIMS)[:-1]]).astype(np.int64)

N_CORES = 8
B_LOC = B // N_CORES          # 2048
NT = 4                        # tiles per core
BT = 512                      # batch per tile
NS = 4                        # subtiles (of 128) per tile
P = 128

FW = 33                       # field width in feature layout (32 emb + bias)
D0 = NF * FW                  # 330 gathered feature columns
NCH = 3                       # k-chunks: [0:128), [128:256), [256:346)
KC = [P, P, 112]              # contraction size per chunk (incl. dense rows)
DROW = 96                     # dense rows begin here within chunk 2 (32-aligned)

F32 = mybir.dt.float32
F32R = mybir.dt.float32r
I32 = mybir.dt.int32

USE_F32R = True               # full-speed PE path; flip to False for exact fp32
MMDT = F32R if USE_F32R else F32

# wall (f32r weight blob) column offsets
W0_O = 0                      # 3 chunks x 256
W1_O = 768                    # 2 chunks x 128
W2_O = 1024                   # 64
WO_O = 1088                   # 1 (rows 0:64)
FM_O = 1089                   # 3 chunks x 34
HV_O = 1191                   # 1 (rows 0:32, value 0.5)
WALL_W = 1192
# aux (f32 blob) column offsets
ID_O = 0                      # identity 128
B1_O = 128                    # 2
B2_O = 130                    # 1
B3_O = 131                    # 1 (rows 0:64)
SC_O = 132                    # 1 (row 0: bo + fm_bias)
AUX_W = 133


# ---------------------------------------------------------------- device code
def _build_nc(reps=1):
    nc = bacc.Bacc("TRN2", target_bir_lowering=False, debug=False)

    tab = nc.dram_tensor("tab", [V, FW], F32, kind="ExternalInput").ap()
    gidx = nc.dram_tensor("gidx", [NT, P, NS * NF], I32, kind="ExternalInput").ap()
    denset = nc.dram_tensor("denset", [NT, DENSE, BT], F32,
                            kind="ExternalInput").ap()
    walld = nc.dram_tensor("walld", [P, WALL_W], MMDT, kind="ExternalInput").ap()
    auxd = nc.dram_tensor("auxd", [P, AUX_W], F32, kind="ExternalInput").ap()
    y = nc.dram_tensor("y", [NT, BT], F32, kind="ExternalOutput").ap()

    from contextlib import ExitStack
    with tile.TileContext(nc) as tc, ExitStack() as ctx:
        wp = ctx.enter_context(tc.tile_pool(name="weights", bufs=1))

        wall = wp.tile([P, WALL_W], MMDT, tag="wall")
        nc.sync.dma_start(out=wall[:], in_=walld[:])
        aux = wp.tile([P, AUX_W], F32, tag="aux")
        nc.sync.dma_start(out=aux[:], in_=auxd[:])
        dummy = wp.tile([1, 1], F32, tag="dummy")

        ident = aux[:, ID_O:ID_O + P]

        def w0(c, o):
            kc = KC[c]
            return wall[0:kc, W0_O + c * MLP[0] + o * P:W0_O + c * MLP[0] + (o + 1) * P]

        def w1(k):
            return wall[:, W1_O + k * MLP[1]:W1_O + (k + 1) * MLP[1]]

        def fmw_s(c):
            return wall[0:KC[c], FM_O + c * 34:FM_O + c * 34 + FW]

        def fmw_q(c):
            return wall[0:KC[c], FM_O + c * 34 + FW:FM_O + c * 34 + 34]

        ip = ctx.enter_context(tc.tile_pool(name="idx", bufs=2))
        gp = ctx.enter_context(tc.tile_pool(name="gather", bufs=2))
        dsp = ctx.enter_context(tc.tile_pool(name="dstage", bufs=2))
        xp = ctx.enter_context(tc.tile_pool(name="xchunks", bufs=6))
        xqp = ctx.enter_context(tc.tile_pool(name="xsq", bufs=2))
        s2p = ctx.enter_context(tc.tile_pool(name="s2", bufs=2))
        hp = ctx.enter_context(tc.tile_pool(name="acts", bufs=6))
        yp = ctx.enter_context(tc.tile_pool(name="yout", bufs=2))

        ps_x = ctx.enter_context(tc.tile_pool(name="ps_x", bufs=2, space="PSUM"))
        ps_s = ctx.enter_context(tc.tile_pool(name="ps_s", bufs=1, space="PSUM"))
        ps_fm = ctx.enter_context(tc.tile_pool(name="ps_fm", bufs=1, space="PSUM"))
        ps_h1 = ctx.enter_context(tc.tile_pool(name="ps_h1", bufs=2, space="PSUM"))
        ps_h2 = ctx.enter_context(tc.tile_pool(name="ps_h2", bufs=1, space="PSUM"))
        ps_h3 = ctx.enter_context(tc.tile_pool(name="ps_h3", bufs=1, space="PSUM"))

        # Absorber instructions: make PE/ACT observe the weight/aux DMA
        # semaphores via single-wait instructions before any real consumer.
        xps_a = ps_x.tile([P, BT], F32, tag="xps")
        nc.tensor.transpose(out=xps_a[:, 0:P], in_=ident, identity=ident)
        nc.tensor.transpose(out=xps_a[:, P:2 * P], in_=wall[:, 0:P].bitcast(F32),
                            identity=ident)
        nc.scalar.copy(dummy[:], aux[0:1, B1_O:B1_O + 1])

        import contextlib
        loop_cm = tc.For_i(0, reps, 1) if reps > 1 else contextlib.nullcontext()
        with loop_cm:
          for t in range(NT):
            idx = ip.tile([P, NS * NF], I32, tag="idx")
            nc.sync.dma_start(out=idx[:], in_=gidx[t])

            G = gp.tile([P, NS * D0], F32, tag="G")
            G3 = G[:].rearrange("p (s f) -> p s f", s=NS)
            # HW indirect DMA gathers one table row per partition per
            # instruction (idx[p, j] -> G[p, s, f]); 40 gathers cover the
            # tile's 512 samples x 10 fields.
            for s in range(NS):
                for f in range(NF):
                    j = s * NF + f
                    nc.gpsimd.indirect_dma_start(
                        out=G3[:, s, f * FW:(f + 1) * FW],
                        out_offset=None,
                        in_=tab,
                        in_offset=bass.IndirectOffsetOnAxis(
                            ap=idx[:, j:j + 1], axis=0),
                    )
            dstage = dsp.tile([DENSE, BT], F32, tag="dstage")
            nc.sync.dma_start(out=dstage[:], in_=denset[t])

            # transpose to feature-major chunks
            Xs = []
            for c in range(NCH):
                w = min(P, D0 - c * P)      # 128 / 128 / 74 gathered cols
                xps = ps_x.tile([P, BT], F32, tag="xps")
                for s in range(NS):
                    nc.tensor.transpose(
                        out=xps[0:w, s * P:(s + 1) * P],
                        in_=G3[:, s, c * P:c * P + w],
                        identity=ident,
                    )
                X = xp.tile([P, BT], MMDT, tag="X")
                if c == 2:
                    # zero the 64:96 band so the K=112 matmul reads no garbage
                    # (f32r memset has no ISA encoding; broadcast-copy zeros
                    # from an all-zero f32r column of the weight blob)
                    nc.vector.tensor_copy(
                        X[64:DROW, :],
                        wall[64:DROW, HV_O:HV_O + 1].to_broadcast([DROW - 64, BT]))
                nc.vector.tensor_copy(X[0:w, :], xps[0:w, :])
                if c == 2:
                    nc.vector.tensor_copy(X[DROW:DROW + DENSE, :], dstage[:])
                Xs.append(X)

            # FM: S rows 0..31 = per-dim field sums, row 32 = bias sum
            sp = ps_s.tile([FW, BT], F32, tag="sp")
            for c in range(NCH):
                nc.tensor.matmul(sp[:], lhsT=fmw_s(c), rhs=Xs[c][0:KC[c], :],
                                 start=(c == 0), stop=(c == NCH - 1))

            fmp = ps_fm.tile([1, BT], F32, tag="fmp")
            for c in range(NCH):
                kc = KC[c]
                Xq = xqp.tile([P, BT], MMDT, tag="Xq")
                nc.vector.tensor_mul(Xq[0:kc, :], Xs[c][0:kc, :], Xs[c][0:kc, :])
                nc.tensor.matmul(fmp[:], lhsT=fmw_q(c), rhs=Xq[0:kc, :],
                                 start=(c == 0), stop=False)
            S2 = s2p.tile([EMB, BT], MMDT, tag="S2")
            nc.scalar.square(S2[:], sp[0:EMB, :])
            nc.tensor.matmul(fmp[:], lhsT=wall[0:EMB, HV_O:HV_O + 1], rhs=S2[:],
                             start=False, stop=False)

            # MLP layer 0: h1[o, b], o in 2 chunks of 128
            h1s = []
            for o in range(2):
                h1p = ps_h1.tile([P, BT], F32, tag="h1p")
                for c in range(NCH):
                    nc.tensor.matmul(h1p[:], lhsT=w0(c, o), rhs=Xs[c][0:KC[c], :],
                                     start=(c == 0), stop=(c == NCH - 1))
                h1 = hp.tile([P, BT], MMDT, tag="h1")
                nc.scalar.activation(h1[:], h1p[:],
                                     mybir.ActivationFunctionType.Relu,
                                     bias=aux[:, B1_O + o:B1_O + o + 1])
                h1s.append(h1)

            # layer 1
            h2p = ps_h2.tile([P, BT], F32, tag="h2p")
            for k in range(2):
                nc.tensor.matmul(h2p[:], lhsT=w1(k), rhs=h1s[k][:],
                                 start=(k == 0), stop=(k == 1))
            h2 = hp.tile([P, BT], MMDT, tag="h2")
            nc.scalar.activation(h2[:], h2p[:],
                                 mybir.ActivationFunctionType.Relu,
                                 bias=aux[:, B2_O:B2_O + 1])

            # layer 2
            h3p = ps_h3.tile([MLP[2], BT], F32, tag="h3p")
            nc.tensor.matmul(h3p[:], lhsT=wall[:, W2_O:W2_O + MLP[2]], rhs=h2[:],
                             start=True, stop=True)
            h3 = hp.tile([MLP[2], BT], MMDT, tag="h3")
            nc.scalar.activation(h3[:], h3p[:],
                                 mybir.ActivationFunctionType.Relu,
                                 bias=aux[0:MLP[2], B3_O:B3_O + 1])

            # output layer into the FM accumulator
            nc.tensor.matmul(fmp[:], lhsT=wall[0:MLP[2], WO_O:WO_O + 1], rhs=h3[:],
                             start=False, stop=True)

            # presig = fmp + bias_sum row; y = sigmoid(presig + (bo + fm_bias))
            bsum = yp.tile([1, BT], F32, tag="bsum")
            nc.vector.tensor_copy(bsum[:], sp[EMB:FW, :])
            pres = yp.tile([1, BT], F32, tag="pres")
            nc.vector.tensor_add(pres[:], fmp[:], bsum[:])
            ysb = yp.tile([1, BT], F32, tag="ysb")
            nc.scalar.activation(ysb[:], pres[:],
                                 mybir.ActivationFunctionType.Sigmoid,
                                 bias=aux[0:1, SC_O:SC_O + 1])
            nc.sync.dma_start(out=y[t:t + 1, :], in_=ysb[:])

    nc.compile()
    return nc


_NC = None


def _get_nc():
    global _NC
    if _NC is None:
        _NC = _build_nc()
    return _NC


# ---------------------------------------------------------------- host prep
def _prep_shared(emb_table, bias_table, fm_bias, Wo, bo,
                 W0, b0, g0, be0, W1, b1, g1, be1, W2, b2, g2, be2):
    inv = np.float32(1.0 / np.sqrt(1.0 + BN_EPS))

    tab = np.empty([V, FW], np.float32)
    tab[:, :EMB] = emb_table
    tab[:, EMB] = bias_table[:, 0]

    def fold(Wl, bl, gl, bel):
        s = (gl * inv).astype(np.float32)
        return (Wl * s[:, None]).astype(np.float32), (bl * s + bel).astype(np.float32)

    W0f, b0f = fold(W0, b0, g0, be0)
    W1f, b1f = fold(W1, b1, g1, be1)
    W2f, b2f = fold(W2, b2, g2, be2)

    # feature permutation: model col 32f+e -> layout row 33f+e; dense -> 330+d
    w0t = np.zeros([NCH * P, MLP[0]], np.float32)
    for f in range(NF):
        w0t[f * FW:f * FW + EMB, :] = W0f[:, f * EMB:(f + 1) * EMB].T
    w0t[2 * P + DROW:2 * P + DROW + DENSE, :] = W0f[:, NF * EMB:].T

    fmw = np.zeros([NCH * P, 34], np.float32)
    for f in range(NF):
        for e in range(EMB):
            fmw[f * FW + e, e] = 1.0       # field-sum matrix
            fmw[f * FW + e, 33] = -0.5     # -0.5 * sum-of-squares mask
        fmw[f * FW + EMB, 32] = 1.0        # bias-sum mask

    wall = np.zeros([P, WALL_W], np.float32)
    for c in range(NCH):
        wall[:, W0_O + c * MLP[0]:W0_O + (c + 1) * MLP[0]] = w0t[c * P:(c + 1) * P]
    for k in range(2):
        wall[:, W1_O + k * MLP[1]:W1_O + (k + 1) * MLP[1]] = \
            W1f.T[k * P:(k + 1) * P]
    wall[:, W2_O:W2_O + MLP[2]] = W2f.T
    wall[0:MLP[2], WO_O] = Wo[0].astype(np.float32)
    for c in range(NCH):
        wall[:, FM_O + c * 34:FM_O + (c + 1) * 34] = fmw[c * P:(c + 1) * P]
    wall[0:EMB, HV_O] = 0.5

    auxa = np.zeros([P, AUX_W], np.float32)
    auxa[:, ID_O:ID_O + P] = np.eye(P, dtype=np.float32)
    for o in range(2):
        auxa[:, B1_O + o] = b0f[o * P:(o + 1) * P]
    auxa[:, B2_O] = b1f
    auxa[0:MLP[2], B3_O] = b2f
    auxa[0, SC_O] = np.float32(bo[0]) + np.float32(fm_bias[0])

    return dict(tab=tab, walld=wall, auxd=auxa)


def _core_inputs(gl_idx, dense_inputs, c):
    lo = c * B_LOC
    idx_c = gl_idx[lo:lo + B_LOC]                      # [2048, 10]
    gidx = (idx_c.reshape(NT, NS, P, NF)
            .transpose(0, 2, 1, 3)                     # [NT, 128, NS, NF]
            .reshape(NT, P, NS * NF))
    dt_ = (dense_inputs[lo:lo + B_LOC]
           .reshape(NT, BT, DENSE)
           .transpose(0, 2, 1))                        # [NT, DENSE, BT]
    return (np.ascontiguousarray(gidx), np.ascontiguousarray(dt_))


def kernel(sparse_inputs, dense_inputs, emb_table, bias_table, fm_bias,
           Wo, bo, W0, b0, g0, be0, W1, b1, g1, be1, W2, b2, g2, be2):
    sparse_inputs = np.asarray(sparse_inputs)
    dense_inputs = np.asarray(dense_inputs, dtype=np.float32)
    args = [np.asarray(a, dtype=np.float32) for a in
            (emb_table, bias_table, fm_bias, Wo, bo,
             W0, b0, g0, be0, W1, b1, g1, be1, W2, b2, g2, be2)]
    shared = _prep_shared(*args)

    gl_idx = (sparse_inputs.astype(np.int64) + OFFSETS[None, :]).astype(np.int32)

    in_maps = []
    for c in range(N_CORES):
        gidx, dt_ = _core_inputs(gl_idx, dense_inputs, c)
        in_maps.append(dict(shared, gidx=gidx, denset=dt_))

    nc = _get_nc()
    res = run_bass_kernel_spmd(nc, in_maps, list(range(N_CORES)),
                               trace=bool(os.environ.get("BASS_TRACE")))
    kernel.last_results = res

    out = np.empty([B], np.float32)
    for c in range(N_CORES):
        out[c * B_LOC:(c + 1) * B_LOC] = res.results[c]["y"].reshape(-1)
    return out

